# revision 1
# baseline (speedup 1.0000x reference)
"""CRF loss (forward-algorithm partition function minus gold path score) on 8
Trainium2 NeuronCores.

Algorithm
---------
In exp space the CRF forward recurrence is linear:

    a_{t+1} = diag(exp(feat_t)) @ exp(transitions) @ a_t

Products of positive matrices contract to rank one extremely fast (measured:
~2e-9 relative after 8 steps, ~5e-2 after 2, for N(0,1) inputs), so the
T=16384 sequential scan splits into 1024 independent chains of CH=16 steps,
each seeded by a W=3 step warmup from a uniform vector.  Each chain tracks its
own log-scale accumulator; the per-chunk log-scale delta d_n = R_n - Q_n is
exact once the chain has converged (the unknown proportionality constant
cancels), and

    logsumexp(alpha_T) = sum_n d_n + T * C_SHIFT.

Each core runs 128 chains in lockstep.  One sync step is a [1024x1024] @
[1024x128] bf16 matvec batch on the PE in a hybrid layout: label blocks 0-5
use a-stationary matmuls (a as weights, E^T streamed at N=512/256, result in
[chain, label] orientation, PE-transposed back via an identity matmul), and
blocks 6-7 use E^T-stationary matmuls that produce [label, chain] directly so
the next step's accumulations (ordered ci = 6,7,0..5) can begin while the
transpose chain for blocks 0-5 drains.  ACT does one exp of the feature tile
per step; DVE applies exp(feat) and evacuates PSUM.  Periodic rescaling
divides by the per-chain column sum (a ones-vector matmul); the division is
deferred into the next step's fp32 multiply so no bf16 rounding enters the
scale bookkeeping.

The gold path score is two flat indirect gathers (offsets precomputed on the
host) plus reductions, kept entirely on GpSimd so the scan engines never queue
behind them.
"""

import numpy as np
import ml_dtypes

import concourse.bass as bass
import concourse.mybir as mybir
import concourse.tile as tile
from concourse import bacc
from concourse.bass_isa import ReduceOp
from concourse.bass_utils import run_bass_kernel_spmd
from concourse.masks import make_identity

DT = mybir.dt
AF = mybir.ActivationFunctionType
OP = mybir.AluOpType

T = 16384
L = 1024
NCORES = 8
TPC = T // NCORES          # rows per core (2048)
CH = 16                    # chunk length (steps per chain)
W = 3                      # warmup steps
SS = W + CH                # sync steps (24)
C = TPC // CH              # chains per core (128)
NB = L // 128              # label blocks (8)
HB = NB // 2               # blocks per PSUM bank (4)
GC = TPC // 128            # gold chunks per core (16)
C_SHIFT = 7.0
START = L - 2

_compiled = {}


def _build():
    nc = bacc.Bacc("TRN2", target_bir_lowering=False, debug=False)

    # feats[s, p, b, m] = P[CH*m + s - W, b*128 + p]  (pre-permuted on host so
    # each per-step DMA is one fully contiguous 512KB block)
    feats = nc.dram_tensor("feats", [SS, 128, NB * C], DT.float32,
                           kind="ExternalInput")
    transT = nc.dram_tensor("transT", [L, L], DT.float32, kind="ExternalInput")
    # flat gather offsets for the gold score: emit terms (into feats) and
    # transition terms (into transT)
    ofs_e = nc.dram_tensor("ofs_e", [128, GC], DT.int32, kind="ExternalInput")
    ofs_t = nc.dram_tensor("ofs_t", [128, GC], DT.int32, kind="ExternalInput")
    mask0 = nc.dram_tensor("mask0", [128, C], DT.bfloat16, kind="ExternalInput")
    init7 = nc.dram_tensor("init7", [128, C], DT.bfloat16, kind="ExternalInput")

    qr = nc.dram_tensor("qr", [2, C], DT.float32, kind="ExternalOutput")
    gold = nc.dram_tensor("gold", [1, GC], DT.float32, kind="ExternalOutput")

    with tile.TileContext(nc) as tc:
        with (
            tc.tile_pool(name="const", bufs=1) as cpool,
            tc.tile_pool(name="tmp", bufs=2) as tmppool,
            tc.tile_pool(name="state", bufs=2) as apool,
            tc.tile_pool(name="feat", bufs=4) as fpool,
            tc.tile_pool(name="small", bufs=2) as spool,
            tc.tile_pool(name="goldp", bufs=2) as gpool,
            tc.tile_pool(name="ps", bufs=1, space="PSUM") as pspool,
            tc.tile_pool(name="ss", bufs=1, space="PSUM") as sspool,
        ):
            # ---------------- prep ----------------
            bias7 = cpool.tile([128, 1], DT.float32)
            nc.gpsimd.memset(bias7[:], -C_SHIFT)

            # E^T tiles, bf16: et[:, ci*L + j] = exp(transT[ci*128 + p, j] - 7)
            # (DMAs spread across idle engine queues so they don't serialize
            # behind the feat stream on sync)
            # chunks 6,7 first: the scan's accumulation order consumes them
            # first, so step 0 starts as early as possible
            et = cpool.tile([128, NB * L], DT.bfloat16)
            tt_all = cpool.tile([128, NB * L], DT.float32)
            for i, ci in enumerate((6, 7, 0, 1, 2, 3, 4, 5)):
                eng = nc.scalar if i % 2 == 0 else nc.gpsimd
                eng.dma_start(tt_all[:, bass.ts(ci, L)],
                              transT[bass.ts(ci, 128), :])
                nc.scalar.activation(et[:, bass.ts(ci, L)],
                                     tt_all[:, bass.ts(ci, L)], AF.Exp,
                                     bias=bias7[:])

            ones_bf = cpool.tile([128, 1], DT.bfloat16)
            nc.gpsimd.memset(ones_bf[:], 1.0)
            col_ones = cpool.tile([1, 128], DT.bfloat16)
            nc.gpsimd.memset(col_ones[:], 1.0)

            ident = cpool.tile([128, 128], DT.bfloat16)
            make_identity(nc, ident[:])

            mask_sb = cpool.tile([128, C], DT.bfloat16)
            nc.gpsimd.dma_start(mask_sb[:], mask0[:])
            init_sb = cpool.tile([128, C], DT.bfloat16)
            nc.gpsimd.dma_start(init_sb[:], init7[:])

            ofse_sb = cpool.tile([128, GC], DT.int32)
            nc.gpsimd.dma_start(ofse_sb[:], ofs_e[:])
            ofst_sb = cpool.tile([128, GC], DT.int32)
            nc.gpsimd.dma_start(ofst_sb[:], ofs_t[:])

            lt = cpool.tile([1, C], DT.float32)
            nc.gpsimd.memset(lt[:], 0.0)

            # initial uniform state (1/1024 is exact in bf16)
            a_cur = apool.tile([128, NB * C], DT.bfloat16, tag="a")
            nc.gpsimd.memset(a_cur[:], 1.0 / L)

            pending_rb = None

            def rescale(a_tile, q_row=False, final=False):
                """Measure per-chain column sums, accumulate log into lt,
                optionally snapshot lt to a qr row; unless final, prepare the
                deferred fp32 division factor broadcast."""
                nonlocal pending_rb
                ssps = sspool.tile([1, C], DT.float32, tag="ss")
                for ci in range(NB):
                    nc.tensor.matmul(ssps[:], ones_bf[:],
                                     a_tile[:, bass.ts(ci, C)],
                                     start=(ci == 0), stop=(ci == NB - 1))
                s_sb = spool.tile([1, C], DT.float32, tag="s")
                nc.vector.tensor_copy(s_sb[:], ssps[:])
                lntmp = spool.tile([1, C], DT.float32, tag="ln")
                nc.scalar.activation(lntmp[:], s_sb[:], AF.Ln)
                nc.vector.tensor_tensor(lt[:], lt[:], lntmp[:], OP.add)
                if q_row:
                    nc.sync.dma_start(qr[0:1, :], lt[:])
                if final:
                    nc.sync.dma_start(qr[1:2, :], lt[:])
                else:
                    # broadcast 1/s across partitions with a K=1 outer
                    # product on the PE (gpsimd is busy with gold gathers)
                    r_sb = spool.tile([1, C], DT.float32, tag="r")
                    nc.vector.reciprocal(r_sb[:], s_sb[:])
                    r_bf = spool.tile([1, C], DT.bfloat16, tag="rbf")
                    nc.vector.tensor_copy(r_bf[:], r_sb[:])
                    rb = sspool.tile([128, C], DT.float32, tag="rb")
                    nc.tensor.matmul(rb[:], col_ones[:], r_bf[:],
                                     start=True, stop=True)
                    pending_rb = rb

            # ---------------- scan ----------------
            for s in range(SS):
                if s == W:
                    # splice in the exact init for chain 0 (data is a no-op
                    # mask on cores 1..7), then measure + Q snapshot
                    for b in range(NB):
                        nc.vector.tensor_tensor(a_cur[:, bass.ts(b, C)],
                                                a_cur[:, bass.ts(b, C)],
                                                mask_sb[:], OP.mult)
                    nc.vector.tensor_tensor(a_cur[:, bass.ts(NB - 1, C)],
                                            a_cur[:, bass.ts(NB - 1, C)],
                                            init_sb[:], OP.add)
                    rescale(a_cur, q_row=True)

                ft = fpool.tile([128, NB * C], DT.float32, tag="ft")
                nc.sync.dma_start(ft[:], feats[s])
                ef = fpool.tile([128, NB * C], DT.float32, tag="ef")
                nc.scalar.activation(ef[:], ft[:], AF.Exp)
                if pending_rb is not None:
                    for b in range(NB):
                        nc.vector.tensor_tensor(ef[:, bass.ts(b, C)],
                                                ef[:, bass.ts(b, C)],
                                                pending_rb[:], OP.mult)
                    pending_rb = None

                # Hybrid matvec. Blocks 0-5: a-stationary (psum in [m, j],
                # PE-transposed back). Blocks 6-7: E-stationary (psum directly
                # in [j, m] — no transpose, so these land early and the next
                # step's accumulations start with ci=6,7 while the
                # transpose+multiply chain for blocks 0-5 drains.
                ci_order = [6, 7, 0, 1, 2, 3, 4, 5]
                a_new = apool.tile([128, NB * C], DT.bfloat16, tag="a")
                u = fpool.tile([128, 6 * C], DT.bfloat16, tag="u")

                psA = pspool.tile([128, 4 * C], DT.float32, tag="psA")
                for i, ci in enumerate(ci_order):
                    nc.tensor.matmul(
                        psA[:], a_cur[:, bass.ts(ci, C)],
                        et[:, ci * L: ci * L + 512],
                        start=(i == 0), stop=(i == NB - 1))
                psB = pspool.tile([128, 2 * C], DT.float32, tag="psB")
                for i, ci in enumerate(ci_order):
                    nc.tensor.matmul(
                        psB[:], a_cur[:, bass.ts(ci, C)],
                        et[:, ci * L + 512: ci * L + 768],
                        start=(i == 0), stop=(i == NB - 1))
                psd = []
                for b in (6, 7):
                    pd = pspool.tile([128, C], DT.float32, tag=f"psd{b}")
                    for i, ci in enumerate(ci_order):
                        nc.tensor.matmul(
                            pd[:],
                            et[:, ci * L + b * 128: ci * L + (b + 1) * 128],
                            a_cur[:, bass.ts(ci, C)],
                            start=(i == 0), stop=(i == NB - 1))
                    psd.append(pd)

                nc.vector.tensor_copy(u[:, 0: 4 * C], psA[:])
                nc.vector.tensor_copy(u[:, 4 * C: 6 * C], psB[:])
                for bi, b in enumerate((6, 7)):
                    nc.vector.tensor_tensor(
                        a_new[:, bass.ts(b, C)], psd[bi][:],
                        ef[:, bass.ts(b, C)], OP.mult)

                p2A = pspool.tile([128, 4 * C], DT.bfloat16, tag="p2A")
                for q in range(4):
                    nc.tensor.transpose(p2A[:, bass.ts(q, 128)],
                                        u[:, bass.ts(q, 128)], ident[:])
                p2B = pspool.tile([128, 2 * C], DT.bfloat16, tag="p2B")
                for q in range(2):
                    nc.tensor.transpose(p2B[:, bass.ts(q, 128)],
                                        u[:, bass.ts(4 + q, 128)], ident[:])
                nc.vector.tensor_tensor(a_new[:, 0: 4 * C], p2A[:],
                                        ef[:, 0: 4 * C], OP.mult)
                nc.vector.tensor_tensor(a_new[:, 4 * C: 6 * C], p2B[:],
                                        ef[:, 4 * C: 6 * C], OP.mult)
                a_cur = a_new

                if s >= W and (s + 1 - W) % 8 == 0:
                    rescale(a_cur, final=(s == SS - 1))

            # ---------------- gold path score ----------------
            feats_flat = bass.AP(feats, 0, [[1, SS * 128 * NB * C], [1, 1]])
            transT_flat = bass.AP(transT, 0, [[1, L * L], [1, 1]])
            # everything here stays on gpsimd so the scan engines (PE/DVE/ACT)
            # never queue behind the gathers
            emit_acc = cpool.tile([128, GC], DT.float32)
            trans_acc = cpool.tile([128, GC], DT.float32)
            for c in range(GC):
                nc.gpsimd.indirect_dma_start(
                    out=emit_acc[:, c:c + 1], out_offset=None, in_=feats_flat,
                    in_offset=bass.IndirectOffsetOnAxis(
                        ap=ofse_sb[:, c:c + 1], axis=0))
                nc.gpsimd.indirect_dma_start(
                    out=trans_acc[:, c:c + 1], out_offset=None, in_=transT_flat,
                    in_offset=bass.IndirectOffsetOnAxis(
                        ap=ofst_sb[:, c:c + 1], axis=0))

            nc.gpsimd.tensor_tensor(emit_acc[:], emit_acc[:], trans_acc[:],
                                    OP.add)
            nc.gpsimd.partition_all_reduce(emit_acc[:], emit_acc[:], 128,
                                           ReduceOp.add)
            nc.gpsimd.dma_start(gold[:], emit_acc[0:1, :])

    nc.compile()
    return nc


def kernel(pred_logits, ref, transitions):
    P = np.ascontiguousarray(np.asarray(pred_logits, dtype=np.float32))
    Tr = np.ascontiguousarray(np.asarray(transitions, dtype=np.float32))
    refv = np.asarray(ref).astype(np.int64).ravel()
    assert P.shape == (T, L) and Tr.shape == (L, L) and refv.shape == (T,)

    if "nc" not in _compiled:
        _compiled["nc"] = _build()
    nc = _compiled["nc"]

    transT_np = np.ascontiguousarray(Tr.T)
    bf16 = ml_dtypes.bfloat16

    in_maps = []
    for k in range(NCORES):
        base = k * TPC
        if k == 0:
            praw_k = np.vstack([np.zeros((W, L), np.float32), P[:TPC]])
        else:
            praw_k = P[base - W: base + TPC]

        # feats[s, p, b, m] = praw_k[CH*m + s, b*128 + p]
        idx = CH * np.arange(C)[None, :] + np.arange(SS)[:, None]  # [SS, C]
        fk = praw_k[idx]                                  # [SS, C, L]
        fk = fk.reshape(SS, C, NB, 128)                   # [s, m, b, p]
        feats_k = np.ascontiguousarray(
            fk.transpose(0, 3, 2, 1)                      # [s, p, b, m]
            .reshape(SS, 128, NB * C).astype(np.float32))

        # gold gather offsets: t_local = c*128 + prow
        rk = refv[base: base + TPC]
        tl = np.arange(TPC)
        s_of_t = W + (tl % CH)
        m_of_t = tl // CH
        eflat = ((s_of_t * 128 + (rk % 128)) * NB + rk // 128) * C + m_of_t
        ofse_k = np.ascontiguousarray(
            eflat.reshape(GC, 128).T.astype(np.int32))
        pv = np.concatenate([[START if k == 0 else refv[base - 1]], rk[:-1]])
        tflat = pv * L + rk
        ofst_k = np.ascontiguousarray(
            tflat.reshape(GC, 128).T.astype(np.int32))

        mask_k = np.ones((128, C), dtype=bf16)
        init_k = np.zeros((128, C), dtype=bf16)
        if k == 0:
            mask_k[:, 0] = 0
            init_k[START - (NB - 1) * 128, 0] = 1.0

        in_maps.append({
            "feats": feats_k, "transT": transT_np,
            "ofs_e": ofse_k, "ofs_t": ofst_k,
            "mask0": mask_k, "init7": init_k,
        })

    res = run_bass_kernel_spmd(nc, in_maps, core_ids=list(range(NCORES)))

    d_sum = 0.0
    gold_sum = 0.0
    for k in range(NCORES):
        qr_k = res.results[k]["qr"].astype(np.float64)
        d_sum += (qr_k[1] - qr_k[0]).sum()
        gold_sum += float(res.results[k]["gold"].astype(np.float64).sum())

    loss = d_sum + T * C_SHIFT - gold_sum
    return np.array([loss], dtype=np.float32)

